# revision 2
# baseline (speedup 1.0000x reference)
"""Trainium2 Bass kernel for causal self-attention (B=2, S=2048, D=1024, H=16).

Sharding: 8 cores = 2 batches x 4 head-groups (4 heads / 256 channels each).
Each core computes the qkv projection for its head block, causal attention for
its 4 heads, and a partial output projection (contraction over its 256
channels). The host sums the 4 bf16 partials per batch and adds b_out.

Schedule (HW-calibrated: matmuls ~73ns/512-wide, ACT exp ~170ns/instr, DVE
reciprocal ~7us/instr -> avoided):
  - ascending q-chunk order; the attention loop for chunk c interleaves the
    projections for chunk c+1 and the out-projection for chunk c-1 as PE
    filler units, overlapping them with the exp/evac dependency chain.
  - softmax denominator is replicated via 64 ones-columns in the AV lhsT
    (av rows 64:128 = denominator), so normalization is Exp(-Ln(denom)) on
    the ACT engine (both funcs in the natural_log_exp_and_others table set,
    pinned to a single LoadActFuncSet) + two DVE multiplies straight into
    aT. No gpsimd broadcast, no SBUF-SBUF DMA, no slow DVE reciprocal.
  - causal mask multiply covers only the 128 columns of the diagonal block
    (one shared lower-triangle tile).
  - PSUM->SBUF evacuations (q/k proj, y out-proj) run on ACT (Identity with
    per-partition bias) to keep DVE off the critical path.
  - consolidated input DMAs in critical-path order (wk, first half-chunk of
    xT, biases, wq, wv, rest); y partials written bf16 (halves out traffic).
"""

import sys

if "/opt/trn_rl_repo" not in sys.path:
    sys.path.insert(0, "/opt/trn_rl_repo")

from collections import deque

import numpy as np
import ml_dtypes

import concourse.mybir as mybir
import concourse.tile as tile
from concourse import bacc

B, S, D, H, DK = 2, 2048, 1024, 16, 64
N_CORES = 8
HPC = 4  # heads per core
DH = HPC * DK  # 256 channels per core
P = 128
QC = 512  # q-chunk width
NQC = S // QC  # 4
NKT = S // P  # 16 k-tiles
DT = D // P  # 8 d-tiles
SCALE = 1.0 / np.sqrt(DK)

BF16 = mybir.dt.bfloat16
F32 = mybir.dt.float32


def build_nc(n_cores: int = N_CORES, repeats: int = 1):
    nc = bacc.Bacc("TRN2", target_bir_lowering=False, debug=False, num_devices=n_cores)
    # The act-table placement pass thrashes between the exp-only and ln-only
    # sets (a ~1.3us LoadActFuncSet per softmax division). All our ACT funcs
    # (Exp, Ln, Identity, Copy) live in natural_log_exp_and_others, so
    # constrain the pass to that single set. Other entries are emptied (never
    # chosen) rather than removed so act_func_set_id indices stay aligned
    # with act_info.json for walrus' lower_act.
    import concourse.bacc as _bacc_mod
    from concourse.hw_specs import get_activation_tables as _orig_gat

    def _gat_one_set(arch):
        t = _orig_gat(arch)
        keep = "natural_log_exp_and_others"
        return {k: (v if k == keep else set()) for k, v in t.items()}

    xT = nc.dram_tensor("xT", [D, S], BF16, kind="ExternalInput")
    wq = nc.dram_tensor("wq", [D, DH], BF16, kind="ExternalInput")
    wk = nc.dram_tensor("wk", [D, DH], BF16, kind="ExternalInput")
    wv = nc.dram_tensor("wv", [D, DH], BF16, kind="ExternalInput")
    wo = nc.dram_tensor("wo", [DH, D], BF16, kind="ExternalInput")
    bq = nc.dram_tensor("bq", [2, P], F32, kind="ExternalInput")
    bk = nc.dram_tensor("bk", [2, P], F32, kind="ExternalInput")
    bv = nc.dram_tensor("bv", [1, DH], F32, kind="ExternalInput")
    y = nc.dram_tensor("y", [S, D], BF16, kind="ExternalOutput")

    with tile.TileContext(nc) as tc:
        for _ in range(repeats):
            _body(nc, tc, xT, wq, wk, wv, wo, bq, bk, bv, y)

    _saved_gat = _bacc_mod.get_activation_tables
    _bacc_mod.get_activation_tables = _gat_one_set
    try:
        nc.compile()
    finally:
        _bacc_mod.get_activation_tables = _saved_gat
    return nc


def _body(nc, tc, xT, wq, wk, wv, wo, bq, bk, bv, y):
    add = mybir.AluOpType.add
    Exp = mybir.ActivationFunctionType.Exp
    Ident = mybir.ActivationFunctionType.Identity
    Ln = mybir.ActivationFunctionType.Ln

    xT_r = xT.ap().rearrange("(dt p) t -> p dt t", p=P)
    wq_r = wq.ap().rearrange("(dt p) c -> p dt c", p=P)
    wk_r = wk.ap().rearrange("(dt p) c -> p dt c", p=P)
    wv_r = wv.ap().rearrange("(dt p) c -> p dt c", p=P)

    with (
        tc.tile_pool(name="const", bufs=1) as const,
        tc.tile_pool(name="work", bufs=2) as work,
        tc.tile_pool(name="psum", bufs=1, space="PSUM") as psum,
    ):
        # ---- persistent SBUF state ----
        xT_sb = const.tile([P, DT, S], BF16)
        wq_sb = const.tile([P, DT, DH], BF16)
        wk_sb = const.tile([P, DT, DH], BF16)
        wv_sb = const.tile([P, DT, DH], BF16)
        wo_sb = const.tile([P, 2, D], BF16)
        bq_sb = const.tile([P, 2], F32)
        bk_sb = const.tile([P, 2], F32)
        bv_bc = const.tile([P, DH], F32)
        qT_sb = const.tile([P, 2, S], BF16)  # [ch within pair, pair, t]
        kT_sb = const.tile([P, 2, S], BF16)
        aT_sb = const.tile([P, 2, S], BF16)
        # V' [t-part, ktile, head, 2*dk]; cols DK:2*DK hold ones so av rows
        # 64:128 come out as 64 replicated copies of the softmax denominator
        vpo = const.tile([P, NKT, HPC, 2 * DK], BF16)
        # lower-triangle mask for the 128-wide diagonal block
        mask_sb = const.tile([P, 1, P], BF16)

        # warm the ACT table while DMAs are in flight. Ln-then-Exp narrows
        # the table-load pass to the single natural_log_exp_and_others set
        # (the only one containing both), so the Exp/Ln mix in the body
        # resolves to ONE LoadActFuncSet instead of thrashing per division.
        warm = work.tile([1, 2], F32, tag="warm", bufs=1)
        nc.vector.memset(warm[:], 1.0)
        nc.scalar.activation(warm[:], warm[:], Ln, scale=1.0)
        nc.scalar.activation(warm[:], warm[:], Exp, scale=1.0)

        nc.vector.memset(vpo[:, :, :, DK : 2 * DK], 1.0)
        nc.vector.memset(mask_sb[:], 1.0)
        nc.gpsimd.affine_select(
            out=mask_sb[:, 0, :],
            in_=mask_sb[:, 0, :],
            compare_op=mybir.AluOpType.is_ge,
            fill=0.0,
            base=0,
            pattern=[[1, P]],
            channel_multiplier=-1,
        )

        # ---- input DMAs, critical-path order, one per tensor/chunk ----
        # first K-proj unit runs on a 256-wide half chunk, so stage wk +
        # the first half of xT chunk 0 first; biases right after (the Q/K
        # evacuations need them ~10us in); wo last (outproj starts ~50us in)
        nc.sync.dma_start(wk_sb[:], wk_r[:])
        nc.sync.dma_start(xT_sb[:, :, 0 : QC // 2], xT_r[:, :, 0 : QC // 2])
        nc.sync.dma_start(xT_sb[:, :, QC // 2 : QC], xT_r[:, :, QC // 2 : QC])
        nc.sync.dma_start(bq_sb[:], bq.ap().rearrange("mt p -> p mt"))
        nc.sync.dma_start(bk_sb[:], bk.ap().rearrange("mt p -> p mt"))
        nc.sync.dma_start(bv_bc[0:1, :], bv.ap())
        nc.gpsimd.partition_broadcast(bv_bc[:], bv_bc[0:1, :])
        nc.sync.dma_start(wq_sb[:], wq_r[:])
        nc.sync.dma_start(wv_sb[:], wv_r[:])
        for c4 in range(1, NQC):
            nc.sync.dma_start(
                xT_sb[:, :, c4 * QC : (c4 + 1) * QC],
                xT_r[:, :, c4 * QC : (c4 + 1) * QC],
            )
        nc.sync.dma_start(wo_sb[:], wo.ap().rearrange("(ht p) e -> p ht e", p=P))

        # ---- PE work units (emitted directly or as attention fillers) ----
        def qk_unit(wsb, bsb, dstT, mt, c, half=None):
            lo = c * QC if half is None else c * QC + half * (QC // 2)
            w = QC if half is None else QC // 2

            def emit():
                ps = psum.tile([P, QC], F32, tag="u", bufs=2, name="qkps")
                for dt in range(DT):
                    nc.tensor.matmul(
                        ps[:, 0:w],
                        lhsT=wsb[:, dt, mt * P : (mt + 1) * P],
                        rhs=xT_sb[:, dt, lo : lo + w],
                        start=(dt == 0),
                        stop=(dt == DT - 1),
                    )
                # evacuation on ACT (DVE is the busier engine on HW)
                nc.scalar.activation(
                    dstT[:, mt, lo : lo + w],
                    ps[:, 0:w],
                    Ident,
                    bias=bsb[:, mt : mt + 1],
                    scale=1.0,
                )

            return emit

        def v_unit(kt):
            def emit():
                ps = psum.tile([P, DH], F32, tag="u", bufs=2, name="vps")
                for dt in range(DT):
                    nc.tensor.matmul(
                        ps[:],
                        lhsT=xT_sb[:, dt, kt * P : (kt + 1) * P],
                        rhs=wv_sb[:, dt, :],
                        start=(dt == 0),
                        stop=(dt == DT - 1),
                    )
                nc.vector.tensor_tensor(
                    vpo[:, kt, :, 0:DK],
                    ps[:].rearrange("p (h j) -> p h j", j=DK),
                    bv_bc[:].rearrange("p (h j) -> p h j", j=DK),
                    add,
                )

            return emit

        def op_unit(tt, ec, tag="u", evac="dve"):
            def emit():
                ps = psum.tile([P, QC], F32, tag=tag, bufs=2, name="yps")
                for ht in range(2):
                    nc.tensor.matmul(
                        ps[:],
                        lhsT=aT_sb[:, ht, tt * P : (tt + 1) * P],
                        rhs=wo_sb[:, ht, ec * QC : (ec + 1) * QC],
                        start=(ht == 0),
                        stop=(ht == 1),
                    )
                ysb = work.tile([P, QC], BF16, tag="y", bufs=4)
                if evac == "dve":
                    nc.vector.tensor_copy(ysb[:], ps[:])
                else:
                    nc.scalar.activation(ysb[:], ps[:], Ident, bias=0.0, scale=1.0)
                nc.sync.dma_start(
                    y.ap()[tt * P : (tt + 1) * P, ec * QC : (ec + 1) * QC],
                    ysb[:],
                )

            return emit

        def proj_units(c, split_first=False):
            u = []
            if split_first:
                u.append(qk_unit(wk_sb, bk_sb, kT_sb, 0, c, half=0))
                u.append(qk_unit(wk_sb, bk_sb, kT_sb, 0, c, half=1))
            else:
                u.append(qk_unit(wk_sb, bk_sb, kT_sb, 0, c))
            u.append(qk_unit(wk_sb, bk_sb, kT_sb, 1, c))
            for mt in range(2):
                u.append(qk_unit(wq_sb, bq_sb, qT_sb, mt, c))
            for kt in range(4 * c, 4 * c + 4):
                u.append(v_unit(kt))
            return u

        def outproj_units(c, tail=False):
            u = []
            for i, (tt, ec) in enumerate(
                (tt, ec) for tt in range(4 * c, 4 * c + 4) for ec in range(2)
            ):
                tag = ("sc" if i % 2 else "u") if tail else "u"
                evac = ("dve" if i % 2 else "act") if tail else "act"
                u.append(op_unit(tt, ec, tag=tag, evac=evac))
            return u

        # ---- attention for one head pair / q-chunk, popping fillers ----
        def av_pair(av, hp, pend, qc, last):
            kt, ex = pend
            cl = max(0, kt - 4 * qc) * P
            for hh in range(2):
                nc.tensor.matmul(
                    av[:, hh, cl:QC],
                    lhsT=vpo[:, kt, 2 * hp + hh, :],
                    rhs=ex[:, hh, cl:QC],
                    start=(kt == 0),
                    stop=last,
                )

        def attention(hp, qc, fillers, fill_credit, fill_rate):
            nkt = 4 * (qc + 1)
            av = psum.tile([P, 2, QC], F32, tag="av", bufs=1, name="av")
            pend = None
            for kt in range(nkt):
                diag = kt - 4 * qc
                cl = max(0, diag) * P
                sc = psum.tile([P, 2, QC], F32, tag="sc", bufs=2, name="sc")
                for hh in range(2):
                    lo, hi = hh * DK, (hh + 1) * DK
                    nc.tensor.matmul(
                        sc[:, hh, cl:QC],
                        lhsT=kT_sb[lo:hi, hp, kt * P : (kt + 1) * P],
                        rhs=qT_sb[lo:hi, hp, qc * QC + cl : (qc + 1) * QC],
                        start=True,
                        stop=True,
                    )
                ex = work.tile([P, 2, QC], BF16, tag="exp", bufs=8)
                nc.scalar.activation(
                    ex[:, :, cl:QC], sc[:, :, cl:QC], Exp, scale=SCALE
                )
                if diag >= 0:
                    nc.vector.tensor_mul(
                        ex[:, :, cl : cl + P],
                        ex[:, :, cl : cl + P],
                        mask_sb[:, 0:1, :].to_broadcast((P, 2, P)),
                    )
                if pend is not None:
                    av_pair(av, hp, pend, qc, last=False)
                pend = (kt, ex)
                fill_credit += fill_rate
                while fillers and fill_credit >= 1.0:
                    fill_credit -= 1.0
                    fillers.popleft()()
            av_pair(av, hp, pend, qc, last=True)

            # softmax division: av rows 64:128 hold the denominator
            # (replicated); reciprocal into a base-0 tile, then normalize
            # straight into aT (bf16).
            qs = slice(qc * QC, (qc + 1) * QC)
            # 1/d = Exp(-Ln(d)) on ACT: both funcs live in the
            # natural_log_exp_and_others table set (no table reload), ACT has
            # slack, and it avoids the 7us-per-instr DVE reciprocal.
            lnd = work.tile([DK, 2, QC], F32, tag="lnd", bufs=2)
            nc.scalar.activation(lnd[:], av[DK:P, :, :], Ln, scale=1.0)
            rec = work.tile([DK, 2, QC], F32, tag="rec", bufs=2)
            nc.scalar.activation(rec[:], lnd[:], Exp, scale=-1.0)
            nc.vector.tensor_mul(aT_sb[0:DK, hp, qs], av[0:DK, 0, :], rec[:, 0, :])
            nc.vector.tensor_mul(aT_sb[DK:P, hp, qs], av[0:DK, 1, :], rec[:, 1, :])
            return fill_credit

        # ---- main pipeline: ascending q-chunks ----
        for u in proj_units(0, split_first=True):
            u()
        for c in range(NQC):
            fillers = deque()
            if c + 1 < NQC:
                fillers.extend(proj_units(c + 1))
            if c >= 1:
                fillers.extend(outproj_units(c - 1))
            reserve = []
            if c == NQC - 1:
                # keep a few units back to fill the PE while the final
                # softmax-division chain (DVE) runs before the tail outproj
                for _ in range(3):
                    if fillers:
                        reserve.append(fillers.pop())
            nkts = 2 * 4 * (c + 1)
            rate = len(fillers) / nkts
            credit = 0.0
            for hp in range(2):
                credit = attention(hp, c, fillers, credit, rate)
            while fillers:
                fillers.popleft()()
            for u in reserve:
                u()
        for u in outproj_units(NQC - 1, tail=True):
            u()


def make_core_inputs(x, w_qkv, b_qkv, w_out, b_out):
    """Shard + preprocess full inputs into 8 per-core input dicts."""
    bf16 = ml_dtypes.bfloat16
    x = np.asarray(x, np.float32)
    w_qkv = np.asarray(w_qkv, np.float32)
    b_qkv = np.asarray(b_qkv, np.float32)
    w_out = np.asarray(w_out, np.float32)

    # per-batch transpose+cast computed once and shared by the 4 cores
    xT_cache = [np.ascontiguousarray(x[b].T).astype(bf16) for b in range(B)]
    in_maps = []
    for c in range(N_CORES):
        b, g = divmod(c, 4)
        sl = slice(g * DH, (g + 1) * DH)
        wq = w_qkv[0 * D + g * DH : 0 * D + (g + 1) * DH]  # [DH, D]
        wk = w_qkv[1 * D + g * DH : 1 * D + (g + 1) * DH]
        wv = w_qkv[2 * D + g * DH : 2 * D + (g + 1) * DH]
        in_maps.append(
            {
                "xT": xT_cache[b],
                "wq": np.ascontiguousarray(wq.T).astype(bf16),
                "wk": np.ascontiguousarray(wk.T).astype(bf16),
                "wv": np.ascontiguousarray(wv.T).astype(bf16),
                "wo": np.ascontiguousarray(w_out[:, sl].T).astype(bf16),
                "bq": b_qkv[0 * D + g * DH : 0 * D + (g + 1) * DH]
                .reshape(2, P)
                .astype(np.float32),
                "bk": b_qkv[1 * D + g * DH : 1 * D + (g + 1) * DH]
                .reshape(2, P)
                .astype(np.float32),
                "bv": b_qkv[2 * D + g * DH : 2 * D + (g + 1) * DH]
                .reshape(1, DH)
                .astype(np.float32),
            }
        )
    return in_maps


def gather_output(results, b_out=None):
    """Sum the 4 per-core partials for each batch (+ b_out)."""
    out = np.empty((B, S, D), np.float32)
    for b in range(B):
        acc = results[4 * b]["y"].astype(np.float32)
        for g in range(1, 4):
            acc = acc + results[4 * b + g]["y"].astype(np.float32)
        out[b] = acc
    if b_out is not None:
        out += np.asarray(b_out, np.float32)
    return out


_NC_CACHE = None


def kernel(x, w_qkv, b_qkv, w_out, b_out):
    global _NC_CACHE
    from concourse.bass_utils import run_bass_kernel_spmd

    if _NC_CACHE is None:
        _NC_CACHE = build_nc()
    in_maps = make_core_inputs(x, w_qkv, b_qkv, w_out, b_out)
    res = run_bass_kernel_spmd(_NC_CACHE, in_maps, core_ids=list(range(N_CORES)))
    return gather_output(res.results, b_out=b_out)
